# revision 45
# baseline (speedup 1.0000x reference)
"""Trainium2 Bass kernel for the AdaptPrompt segment-reduce problem.

Computation (see reference):
    counts/centers/delta = per-class segment means over 10000 few-shot rows
    xr = Q1_x[remaining_idxes]                       # [190000, 256] gather
    sim = softmax(normalize(xr) @ normalize(centers).T)
    out = xr + sim @ delta

Key observation: the per-row map f(x) = x + softmax(x_n @ c_n.T) @ delta
commutes with the row gather, so each core computes f on its contiguous
25000-row table slice (fully sequential DMA, no SWDGE descriptor
generation, no indirect gather) and the host applies remaining_idxes as
the final unshard step (mirror of the baseline's host-side scatter).

Distribution over 8 NeuronCores:
  - table rows sharded contiguously, 25000 rows/core (padded to 25088)
  - few-shot phase replicated on every core (10000 rows, fp8, one-hot
    DoubleRow matmul segment sums) -- avoids the AllReduce, whose
    barrier+trigger latency (~88us measured on HW) would dominate
  - host pre-normalizes rows and uploads x-hat TRANSPOSED [2,128,25088]
    fp8e4 (scaled x16) so the similarity matmul needs no on-device
    transposes; column order is G-interleaved so the row-major x/out
    tiles move as G*512B DMA descriptors

Per-core device pipeline (~34.6MB HBM traffic, all engines balanced):
  - fs: 10 batched fp8 tile loads, one-hot pairs reduced by DoubleRow
    matmuls (0.5 cy/row); counts accumulated on the DVE
  - stats: counts recip, centers/delta means, center normalize, cn^T
  - main, per 512 rows: one DoubleRow matmul qq = cnT.T @ xhatT
    (PSUM [16,512]), ACT exp(qq/256), fo = e@[delta|1] in paired
    [128,2,512] PSUM tiles (ones column = softmax denominator, one
    strided DVE reciprocal per pair), finalize out = fo*rinv + x split
    ~3/8 ACT-scale + bf16-add / ~5/8 fused DVE scalar_tensor_tensor
"""

import os
from contextlib import ExitStack

import numpy as np

import concourse.bass as bass
import concourse.mybir as mybir
import concourse.tile as tile
from concourse.bacc import Bacc

DT = mybir.dt
ALU = mybir.AluOpType
ACTF = mybir.ActivationFunctionType

CORES = 8
N, D, NUM = 200000, 256, 16
S, R = 10000, 190000
SLICE = N // CORES            # 25000 table rows per core
RT = 196                      # row tiles per core (196*128 = 25088)
R_PAD = RT * 128              # 25088
S_TILES = 80                  # few-shot tiles (80*128 = 10240 >= 10000)
S_PAD = S_TILES * 128         # 10240
BLKS = [2048] * 12 + [512]    # main-loop block sizes (sum = 25088)
G = 8                         # rows packed per (partition, slot) -> 4KB DMA
                              # (the 512-row tail block falls back to G=4)
XS = 16.0                     # fp8 pre-scale on xhat and cn (qq scaled XS^2)


def _emit_recip(nc, pool, x_ap, shape, tag):
    """1/x via integer-magic seed + Newton steps (plain DVE ops only)."""
    seed_i = pool.tile(shape, DT.int32, name=f"{tag}_si")
    nc.vector.tensor_scalar(
        out=seed_i[:], in0=x_ap.bitcast(DT.int32), scalar1=-1, scalar2=0x7EF477D5,
        op0=ALU.mult, op1=ALU.add)
    y = pool.tile(shape, DT.float32, name=f"{tag}_y")
    nc.vector.tensor_copy(y[:], seed_i[:].bitcast(DT.float32))
    for it in range(2):
        e = pool.tile(shape, DT.float32, name=f"{tag}_e{it}")
        nc.vector.tensor_tensor(out=e[:], in0=x_ap, in1=y[:], op=ALU.mult)
        nc.vector.tensor_scalar(
            out=e[:], in0=e[:], scalar1=-1.0, scalar2=2.0,
            op0=ALU.mult, op1=ALU.add)
        nc.vector.tensor_tensor(out=y[:], in0=y[:], in1=e[:], op=ALU.mult)
    return y


def _emit_rsqrt(nc, pool, x_ap, shape, tag):
    """1/sqrt(x) via 0x5f3759df seed + Newton steps, DVE-only."""
    seed_i = pool.tile(shape, DT.int32, name=f"{tag}_si")
    nc.vector.tensor_scalar(
        out=seed_i[:], in0=x_ap.bitcast(DT.int32), scalar1=1, scalar2=None,
        op0=ALU.arith_shift_right)
    nc.vector.tensor_scalar(
        out=seed_i[:], in0=seed_i[:], scalar1=-1, scalar2=0x5F3759DF,
        op0=ALU.mult, op1=ALU.add)
    y = pool.tile(shape, DT.float32, name=f"{tag}_y")
    nc.vector.tensor_copy(y[:], seed_i[:].bitcast(DT.float32))
    for it in range(2):
        t1 = pool.tile(shape, DT.float32, name=f"{tag}_t{it}")
        nc.vector.tensor_tensor(out=t1[:], in0=y[:], in1=y[:], op=ALU.mult)
        nc.vector.tensor_tensor(out=t1[:], in0=x_ap, in1=t1[:], op=ALU.mult)
        nc.vector.tensor_scalar(
            out=t1[:], in0=t1[:], scalar1=-0.5, scalar2=1.5,
            op0=ALU.mult, op1=ALU.add)
        nc.vector.tensor_tensor(out=y[:], in0=y[:], in1=t1[:], op=ALU.mult)
    return y


def build_nc():
    nc = Bacc(target_bir_lowering=False, num_devices=CORES)

    # x-hat transposed (fp8, host-scaled by XS): [h, p, c] holds
    # XS*xhat[pi(c), h*128+p] where pi is the G-interleave permutation that
    # makes the row-major x/out DMA descriptors G*512B long.
    xhT = nc.declare_dram_parameter("xhT", [2, 128, R_PAD], DT.float8e4,
                                    isOutput=False)
    xraw = nc.declare_dram_parameter("xraw", [R_PAD, D], DT.bfloat16,
                                     isOutput=False)
    # few-shot rows [x1 | x2], fp8, partition-major ([p, t] holds row
    # t*128+p), replicated to every core
    x12f = nc.declare_dram_parameter("x12f", [128, S_TILES, 2 * D],
                                     DT.float8e4, isOutput=False)
    yf = nc.declare_dram_parameter("yf", [128, S_TILES], DT.float32,
                                   isOutput=False)
    out = nc.declare_dram_parameter("out", [R_PAD, D], DT.bfloat16,
                                    isOutput=True)

    with tile.TileContext(nc) as tc, ExitStack() as ctx:
        cpool = ctx.enter_context(tc.tile_pool(name="const", bufs=1))

        # ---- constants ----
        ident_f = cpool.tile([128, 128], DT.float32)
        from concourse.masks import make_identity
        make_identity(nc, ident_f[:])
        iota_i = cpool.tile([128, NUM], DT.int32)
        nc.gpsimd.iota(iota_i[:], pattern=[[1, NUM]], base=0, channel_multiplier=0)
        iota_f = cpool.tile([128, 1, NUM], DT.float32)
        nc.vector.tensor_copy(iota_f[:, 0, :], iota_i[:])
        ones_p = cpool.tile([128, 2, 1], DT.float8e4)
        nc.vector.memset(ones_p[:], 1.0)
        yf_sb = cpool.tile([128, S_TILES, 1], DT.float32)
        nc.sync.dma_start(out=yf_sb[:, :, 0], in_=yf[:, :])

        # ---- phase 1: few-shot per-class segment sums (replicated) ----
        cnT_sb = cpool.tile([128, 2, NUM], DT.float8e4)
        delta_bf = cpool.tile([NUM, D + 1], DT.bfloat16)
        # few-shot tiles loaded in batches of 8 (fewer DMA issues: the Sync
        # engine spends ~800ns per dma_start) and reduced two tiles per
        # DoubleRow fp8 matmul (0.5 cycles/row)
        FB = 8
        FS_BATCHES = [(b * FB, min(FB, S_TILES - b * FB))
                      for b in range((S_TILES + FB - 1) // FB)]
        NPAIR = S_TILES // 2
        with tc.tile_pool(name="fsp", bufs=1, space="PSUM") as fsps, \
             tc.tile_pool(name="fs", bufs=10) as fsp:
            cs_ds_ps = fsps.tile([NUM, 2 * D], DT.float32, name="cs_ds_ps")
            cnt_ps = fsps.tile([NUM, 1], DT.float32, name="cnt_ps")
            # warm the PE pstate while the first few-shot tiles stream in
            wlhs = fsp.tile([128, 2, NUM], DT.float8e4, name="wlhs")
            nc.vector.memset(wlhs[:], 1.0)
            wrhs = fsp.tile([128, 2, 512], DT.float8e4, name="wrhs")
            nc.vector.memset(wrhs[:], 1.0)
            warm_ps = fsps.tile([NUM, 512], DT.float32, name="warm_ps")
            for _ in range(4):
                nc.tensor.matmul(warm_ps[:], lhsT=wlhs[:], rhs=wrhs[:],
                                 start=True, stop=True,
                                 perf_mode=mybir.MatmulPerfMode.DoubleRow)
            # counts: accumulate the one-hots on the DVE (keeps the serial
            # PE weight-load/matmul chain to one matmul per tile pair)
            oh_acc = cpool.tile([128, FB, NUM], DT.float32)
            nc.vector.memset(oh_acc[:], 0.0)
            for bt, bn in FS_BATCHES:
                fs_b = fsp.tile([128, bn, 2 * D], DT.float8e4, name="fs_b")
                nc.sync.dma_start(out=fs_b[:], in_=x12f[:, bt:bt + bn, :])
                # one-hot labels for the whole batch in a single DVE op
                oh_b = fsp.tile([128, bn, NUM], DT.float8e4, name="oh_b")
                nc.vector.tensor_tensor(
                    out=oh_b[:],
                    in0=yf_sb[:, bt:bt + bn, :].to_broadcast([128, bn, NUM]),
                    in1=iota_f[:].to_broadcast([128, bn, NUM]),
                    op=ALU.is_equal)
                nc.vector.tensor_tensor(
                    out=oh_acc[:, 0:bn, :], in0=oh_acc[:, 0:bn, :],
                    in1=oh_b[:], op=ALU.add)
                for k in range(0, bn, 2):
                    t = bt + k
                    st, sp = (t == 0), (t == S_TILES - 2)
                    nc.tensor.matmul(
                        cs_ds_ps[:], lhsT=oh_b[:, k:k + 2, :],
                        rhs=fs_b[:, k:k + 2, :], start=st, stop=sp,
                        perf_mode=mybir.MatmulPerfMode.DoubleRow)
            # fold the FB slots, then one [128,16]x[128,1] matmul -> counts
            oh_slot = cpool.tile([128, NUM, 1], DT.float32)
            nc.vector.tensor_reduce(
                out=oh_slot[:],
                in_=oh_acc[:].rearrange("p s c -> p c s"),
                axis=mybir.AxisListType.X, op=ALU.add)
            ones_f = cpool.tile([128, 1], DT.float32)
            nc.vector.memset(ones_f[:], 1.0)
            nc.tensor.matmul(cnt_ps[:], lhsT=oh_slot[:, :, 0],
                             rhs=ones_f[:], start=True, stop=True)

            # ---- phase 2: class stats (all on 16 partitions) ----
            sums = cpool.tile([NUM, 2 * D], DT.float32)
            nc.vector.tensor_copy(sums[:], cs_ds_ps[:])
            cnt_sb = cpool.tile([NUM, 1], DT.float32)
            nc.vector.tensor_copy(cnt_sb[:], cnt_ps[:])

        rc = _emit_recip(nc, cpool, cnt_sb[:], [NUM, 1], "rc")
        centers = cpool.tile([NUM, D], DT.float32)
        nc.vector.tensor_scalar_mul(centers[:], sums[:, 0:D], rc[:])
        dsum = cpool.tile([NUM, D], DT.float32)
        nc.vector.tensor_tensor(
            out=dsum[:], in0=sums[:, D:2 * D], in1=sums[:, 0:D], op=ALU.subtract)
        nc.vector.tensor_scalar_mul(delta_bf[:, 0:D], dsum[:], rc[:])
        nc.vector.memset(delta_bf[:, D:D + 1], 1.0)
        csq = cpool.tile([NUM, D], DT.float32)
        nc.vector.tensor_tensor(
            out=csq[:], in0=centers[:], in1=centers[:], op=ALU.mult)
        csum = cpool.tile([NUM, 1], DT.float32)
        nc.vector.tensor_reduce(
            out=csum[:], in_=csq[:], axis=mybir.AxisListType.X, op=ALU.add)
        cinv = _emit_rsqrt(nc, cpool, csum[:], [NUM, 1], "cinv")
        # cn scaled by XS to keep fp8 values in the normal range; the
        # XS^2 factor on qq is undone by the exp scale below
        cinv16 = cpool.tile([NUM, 1], DT.float32)
        nc.vector.tensor_scalar(out=cinv16[:], in0=cinv[:], scalar1=XS,
                                scalar2=None, op0=ALU.mult)
        cn_f = cpool.tile([NUM, D], DT.float32)
        nc.vector.tensor_scalar_mul(cn_f[:], centers[:], cinv16[:])
        with tc.tile_pool(name="cnp", bufs=1, space="PSUM") as cnps:
            for h in range(2):
                tpc = cnps.tile([128, NUM], DT.float32, name=f"tpc{h}")
                nc.tensor.transpose(
                    tpc[:], in_=cn_f[:, h * 128:(h + 1) * 128],
                    identity=ident_f[0:NUM, 0:NUM])
                nc.vector.tensor_copy(cnT_sb[:, h, :], tpc[:])

        # ---- phase 3: main loop over table row blocks ----
        with tc.tile_pool(name="mi", bufs=8) as mpool, \
             tc.tile_pool(name="mo", bufs=5) as opool, \
             tc.tile_pool(name="mt", bufs=6) as tpool, \
             tc.tile_pool(name="mq", bufs=2, space="PSUM") as qps, \
             tc.tile_pool(name="mf", bufs=3, space="PSUM") as fps:
            sched = [(i * 2048, 2048) for i in range(12)] + [(R_PAD - 512, 512)]
            for off, nrows in sched:
                nt = nrows // 128
                gb = min(G, nt)          # tail block packs fewer rows/slot
                ns = nt // gb
                xgT_blk = mpool.tile([128, 2, nrows], DT.float8e4, name="xgT_blk")
                for h in range(2):
                    nc.sync.dma_start(out=xgT_blk[:, h, :],
                                      in_=xhT[h, :, off:off + nrows])
                x_blk = mpool.tile([128, ns, gb * D], DT.bfloat16, name="x_blk")
                nsplit = 2 if ns % 2 == 0 else 1
                nh = nrows // nsplit
                nsh = ns // nsplit
                for hh in range(nsplit):
                    nc.sync.dma_start(
                        out=x_blk[:, hh * nsh:(hh + 1) * nsh, :],
                        in_=xraw[off + hh * nh:off + (hh + 1) * nh, :].rearrange(
                            "(js p g) d -> p js (g d)", p=128, g=gb))
                out_blk = opool.tile([128, ns, gb * D], DT.bfloat16,
                                     name="out_blk")
                # softmax 1/denominator as exp(-ln(den)) on the Scalar
                # engine (ln+exp+copy live in one ACT table set) -- keeps
                # the DVE down to one fused op per row tile.
                for sb in range(nrows // 512):
                    qq = qps.tile([NUM, 512], DT.float32, name="qq")
                    nc.tensor.matmul(
                        qq[:], lhsT=cnT_sb[:],
                        rhs=xgT_blk[:, :, sb * 512:(sb + 1) * 512],
                        start=True, stop=True,
                        perf_mode=mybir.MatmulPerfMode.DoubleRow)
                    e4 = tpool.tile([NUM, 512], DT.bfloat16, name="e4")
                    nc.scalar.activation(out=e4[:], in_=qq[:], func=ACTF.Exp,
                                         scale=1.0 / (XS * XS))
                    # two fo tiles share one 2-bank PSUM allocation so a
                    # single strided DVE reciprocal covers both softmax
                    # denominators (no per-tile copies on any engine)
                    fos, rses = [], []
                    for half in range(2):
                        fo2 = fps.tile([128, 2, 512], DT.float32, name="fo2")
                        for k in range(2):
                            t4 = half * 2 + k
                            nc.tensor.matmul(
                                fo2[:, k, 0:D + 1],
                                lhsT=e4[:, t4 * 128:(t4 + 1) * 128],
                                rhs=delta_bf[:], start=True, stop=True)
                        rse2 = tpool.tile([128, 2], DT.float32,
                                          name=f"rse{half}")
                        nc.vector.reciprocal(rse2[:], fo2[:, :, D])
                        fos.append(fo2)
                        rses.append(rse2)
                    # finalize: tiles 0,2 via fused DVE op; tiles 1,3
                    # scaled on the Scalar engine, then ONE strided bf16
                    # add covers both (2x DVE mode, halves op overhead)
                    sc2 = tpool.tile([128, 2, D], DT.bfloat16, name="sc2")
                    for t4 in (0, 2):
                        j = sb * 4 + t4
                        js, g = j // gb, j % gb
                        nc.vector.scalar_tensor_tensor(
                            out=out_blk[:, js, g * D:(g + 1) * D],
                            in0=fos[t4 // 2][:, t4 % 2, 0:D],
                            scalar=rses[t4 // 2][:, t4 % 2:t4 % 2 + 1],
                            in1=x_blk[:, js, g * D:(g + 1) * D],
                            op0=ALU.mult, op1=ALU.add)
                    for i, t4 in enumerate((1, 3)):
                        nc.scalar.mul(sc2[:, i, :], fos[t4 // 2][:, t4 % 2, 0:D],
                                      rses[t4 // 2][:, t4 % 2:t4 % 2 + 1])
                    j1 = sb * 4 + 1
                    js1, g1 = j1 // gb, j1 % gb
                    # tiles 1 and 3 sit two D-slots apart in the same js slot
                    xs_ap = x_blk[:, js1, :].rearrange(
                        "p (s d) -> p s d", d=D)[:, g1:g1 + 3:2, :]
                    os_ap = out_blk[:, js1, :].rearrange(
                        "p (s d) -> p s d", d=D)[:, g1:g1 + 3:2, :]
                    nc.vector.tensor_tensor(
                        out=os_ap, in0=sc2[:], in1=xs_ap, op=ALU.add)
                # stores go out the gpsimd software-DGE ring (Q0) so they
                # never head-of-line block the Q1 prefetch stream
                for hh in range(nsplit):
                    nc.gpsimd.dma_start(
                        out=out[off + hh * nh:off + (hh + 1) * nh, :].rearrange(
                            "(js p g) d -> p js (g d)", p=128, g=gb),
                        in_=out_blk[:, hh * nsh:(hh + 1) * nsh, :])
    nc.finalize()
    return nc


def _shard_inputs(Q1_x, Q2_x, Q1_y, selected_idxes, remaining_idxes):
    """Host-side sharding/layout prep (slicing, normalize, transpose, cast)."""
    import ml_dtypes
    bf16 = ml_dtypes.bfloat16
    fp8 = ml_dtypes.float8_e4m3

    Q1_x = np.asarray(Q1_x, dtype=np.float32)
    Q2_x = np.asarray(Q2_x, dtype=np.float32)
    y = np.asarray(Q1_y).astype(np.float32)
    sel = np.asarray(selected_idxes).astype(np.int64)

    # few-shot block, partition-major, replicated to every core
    x12 = np.zeros((S_PAD, 2 * D), dtype=np.float32)
    x12[:S, 0:D] = Q1_x[sel]
    x12[:S, D:2 * D] = Q2_x[sel]
    x12 = np.ascontiguousarray(
        x12.reshape(S_TILES, 128, 2 * D).transpose(1, 0, 2)).astype(fp8)
    yv = np.full((S_PAD,), -1.0, dtype=np.float32)
    yv[:S] = y[sel]
    yf = np.ascontiguousarray(yv.reshape(S_TILES, 128).T)  # [128, S_TILES]

    norms = np.maximum(np.sqrt((Q1_x * Q1_x).sum(axis=1, keepdims=True)), 1e-8)
    xhat = Q1_x * (np.float32(XS) / norms)

    # xhatT column order: pi(t*128+q) = (t//G)*G*128 + q*G + (t%G), so the
    # row-major x/out tiles pack G consecutive DRAM rows per partition slot
    t = np.arange(R_PAD)
    tt, q = t // 128, t % 128
    pi = (tt // G) * G * 128 + q * G + (tt % G)
    tail = tt >= 192            # 512-row tail block uses G=4
    pi[tail] = 24576 + q[tail] * 4 + (tt[tail] - 192)

    in_maps = []
    for c in range(CORES):
        sl = slice(c * SLICE, (c + 1) * SLICE)
        xh_pad = np.zeros((R_PAD, D), dtype=np.float32)
        xh_pad[:SLICE] = xhat[sl]
        xhT = np.ascontiguousarray(
            xh_pad[pi].T.reshape(2, 128, R_PAD)).astype(fp8)
        xr_pad = np.zeros((R_PAD, D), dtype=np.float32)
        xr_pad[:SLICE] = Q1_x[sl]
        in_maps.append({
            "xhT": xhT,
            "xraw": xr_pad.astype(bf16),
            "x12f": x12,
            "yf": yf,
        })
    return in_maps


def kernel(Q1_x, Q2_x, Q1_y, selected_idxes, remaining_idxes, num, _bench=None):
    from concourse.bass_utils import run_bass_kernel_spmd

    in_maps = _shard_inputs(Q1_x, Q2_x, Q1_y, selected_idxes, remaining_idxes)
    nc = build_nc()
    kwargs = dict(_bench or {})
    res = run_bass_kernel_spmd(nc, in_maps, core_ids=list(range(CORES)), **kwargs)
    full = np.concatenate(
        [np.asarray(res.results[c]["out"][:SLICE]) for c in range(CORES)], axis=0)
    rem = np.asarray(remaining_idxes).astype(np.int64)
    out = full[rem].astype(np.float32)
    if _bench is not None:
        kernel.last_results = res
    return out


# revision 50
# speedup vs baseline: 1.0105x; 1.0105x over previous
"""Trainium2 Bass kernel for the AdaptPrompt segment-reduce problem.

Computation (see reference):
    counts/centers/delta = per-class segment means over 10000 few-shot rows
    xr = Q1_x[remaining_idxes]                       # [190000, 256] gather
    sim = softmax(normalize(xr) @ normalize(centers).T)
    out = xr + sim @ delta

Key observation: the per-row map f(x) = x + softmax(x_n @ c_n.T) @ delta
commutes with the row gather, so each core computes f on its contiguous
25000-row table slice (fully sequential DMA, no SWDGE descriptor
generation, no indirect gather) and the host applies remaining_idxes as
the final unshard step (mirror of the baseline's host-side scatter).

Distribution over 8 NeuronCores:
  - table rows sharded contiguously, 25000 rows/core (padded to 25088)
  - few-shot phase replicated on every core (10000 rows, fp8, one-hot
    DoubleRow matmul segment sums) -- avoids the AllReduce, whose
    barrier+trigger latency (~88us measured on HW) would dominate
  - host pre-normalizes rows and uploads x-hat TRANSPOSED [2,128,25088]
    fp8e4 (scaled x16) so the similarity matmul needs no on-device
    transposes; column order is G-interleaved so the row-major x/out
    tiles move as G*512B DMA descriptors

Per-core device pipeline (~34.6MB HBM traffic, all engines balanced):
  - fs: 10 batched fp8 tile loads, one-hot pairs reduced by DoubleRow
    matmuls (0.5 cy/row); counts accumulated on the DVE
  - stats: counts recip, centers/delta means, center normalize, cn^T
  - main, per 512 rows: one DoubleRow matmul qq = cnT.T @ xhatT
    (PSUM [16,512]), ACT exp(qq/256), fo = e@[delta|1] in paired
    [128,2,512] PSUM tiles (ones column = softmax denominator, one
    strided DVE reciprocal per pair), finalize out = fo*rinv + x split
    ~3/8 ACT-scale + bf16-add / ~5/8 fused DVE scalar_tensor_tensor
"""

import os
from contextlib import ExitStack

import numpy as np

import concourse.bass as bass
import concourse.mybir as mybir
import concourse.tile as tile
from concourse.bacc import Bacc

DT = mybir.dt
ALU = mybir.AluOpType
ACTF = mybir.ActivationFunctionType

CORES = 8
N, D, NUM = 200000, 256, 16
S, R = 10000, 190000
SLICE = N // CORES            # 25000 table rows per core
RT = 196                      # row tiles per core (196*128 = 25088)
R_PAD = RT * 128              # 25088
S_TILES = 80                  # few-shot tiles (80*128 = 10240 >= 10000)
S_PAD = S_TILES * 128         # 10240
BLKS = [2048] * 12 + [512]    # main-loop block sizes (sum = 25088)
G = 8                         # rows packed per (partition, slot) -> 4KB DMA
                              # (the 512-row tail block falls back to G=4)
XS = 16.0                     # fp8 pre-scale on xhat and cn (qq scaled XS^2)


def _emit_recip(nc, pool, x_ap, shape, tag):
    """1/x via integer-magic seed + Newton steps (plain DVE ops only)."""
    seed_i = pool.tile(shape, DT.int32, name=f"{tag}_si")
    nc.vector.tensor_scalar(
        out=seed_i[:], in0=x_ap.bitcast(DT.int32), scalar1=-1, scalar2=0x7EF477D5,
        op0=ALU.mult, op1=ALU.add)
    y = pool.tile(shape, DT.float32, name=f"{tag}_y")
    nc.vector.tensor_copy(y[:], seed_i[:].bitcast(DT.float32))
    for it in range(2):
        e = pool.tile(shape, DT.float32, name=f"{tag}_e{it}")
        nc.vector.tensor_tensor(out=e[:], in0=x_ap, in1=y[:], op=ALU.mult)
        nc.vector.tensor_scalar(
            out=e[:], in0=e[:], scalar1=-1.0, scalar2=2.0,
            op0=ALU.mult, op1=ALU.add)
        nc.vector.tensor_tensor(out=y[:], in0=y[:], in1=e[:], op=ALU.mult)
    return y


def _emit_rsqrt(nc, pool, x_ap, shape, tag):
    """1/sqrt(x) via 0x5f3759df seed + Newton steps, DVE-only."""
    seed_i = pool.tile(shape, DT.int32, name=f"{tag}_si")
    nc.vector.tensor_scalar(
        out=seed_i[:], in0=x_ap.bitcast(DT.int32), scalar1=1, scalar2=None,
        op0=ALU.arith_shift_right)
    nc.vector.tensor_scalar(
        out=seed_i[:], in0=seed_i[:], scalar1=-1, scalar2=0x5F3759DF,
        op0=ALU.mult, op1=ALU.add)
    y = pool.tile(shape, DT.float32, name=f"{tag}_y")
    nc.vector.tensor_copy(y[:], seed_i[:].bitcast(DT.float32))
    for it in range(2):
        t1 = pool.tile(shape, DT.float32, name=f"{tag}_t{it}")
        nc.vector.tensor_tensor(out=t1[:], in0=y[:], in1=y[:], op=ALU.mult)
        nc.vector.tensor_tensor(out=t1[:], in0=x_ap, in1=t1[:], op=ALU.mult)
        nc.vector.tensor_scalar(
            out=t1[:], in0=t1[:], scalar1=-0.5, scalar2=1.5,
            op0=ALU.mult, op1=ALU.add)
        nc.vector.tensor_tensor(out=y[:], in0=y[:], in1=t1[:], op=ALU.mult)
    return y


def build_nc():
    nc = Bacc(target_bir_lowering=False, num_devices=CORES)

    # x-hat transposed (fp8, host-scaled by XS): [h, p, c] holds
    # XS*xhat[pi(c), h*128+p] where pi is the G-interleave permutation that
    # makes the row-major x/out DMA descriptors G*512B long.
    xhT = nc.declare_dram_parameter("xhT", [2, 128, R_PAD], DT.float8e4,
                                    isOutput=False)
    xraw = nc.declare_dram_parameter("xraw", [R_PAD, D], DT.bfloat16,
                                     isOutput=False)
    # few-shot rows [x1 | x2], fp8, partition-major ([p, t] holds row
    # t*128+p), replicated to every core
    x12f = nc.declare_dram_parameter("x12f", [128, S_TILES, 2 * D],
                                     DT.float8e4, isOutput=False)
    yf = nc.declare_dram_parameter("yf", [128, S_TILES], DT.float32,
                                   isOutput=False)
    out = nc.declare_dram_parameter("out", [R_PAD, D], DT.bfloat16,
                                    isOutput=True)

    with tile.TileContext(nc) as tc, ExitStack() as ctx:
        cpool = ctx.enter_context(tc.tile_pool(name="const", bufs=1))

        # ---- constants ----
        ident_f = cpool.tile([128, 128], DT.float32)
        from concourse.masks import make_identity
        make_identity(nc, ident_f[:])
        iota_i = cpool.tile([128, NUM], DT.int32)
        nc.gpsimd.iota(iota_i[:], pattern=[[1, NUM]], base=0, channel_multiplier=0)
        iota_f = cpool.tile([128, 1, NUM], DT.float32)
        nc.vector.tensor_copy(iota_f[:, 0, :], iota_i[:])
        ones_p = cpool.tile([128, 2, 1], DT.float8e4)
        nc.vector.memset(ones_p[:], 1.0)
        yf_sb = cpool.tile([128, S_TILES, 1], DT.float32)
        nc.sync.dma_start(out=yf_sb[:, :, 0], in_=yf[:, :])

        # ---- phase 1: few-shot per-class segment sums (replicated) ----
        cnT_sb = cpool.tile([128, 2, NUM], DT.float8e4)
        delta_bf = cpool.tile([NUM, D + 1], DT.bfloat16)
        # few-shot tiles loaded in batches of 8 (fewer DMA issues: the Sync
        # engine spends ~800ns per dma_start) and reduced two tiles per
        # DoubleRow fp8 matmul (0.5 cycles/row)
        FB = 16
        FS_BATCHES = [(b * FB, min(FB, S_TILES - b * FB))
                      for b in range((S_TILES + FB - 1) // FB)]
        NPAIR = S_TILES // 2
        with tc.tile_pool(name="fsp", bufs=1, space="PSUM") as fsps, \
             tc.tile_pool(name="fs", bufs=5) as fsp:
            cs_ds_ps = fsps.tile([NUM, 2 * D], DT.float32, name="cs_ds_ps")
            cnt_ps = fsps.tile([NUM, 1], DT.float32, name="cnt_ps")
            # warm the PE pstate while the first few-shot tiles stream in
            wlhs = fsp.tile([128, 2, NUM], DT.float8e4, name="wlhs")
            nc.vector.memset(wlhs[:], 1.0)
            wrhs = fsp.tile([128, 2, 512], DT.float8e4, name="wrhs")
            nc.vector.memset(wrhs[:], 1.0)
            warm_ps = fsps.tile([NUM, 512], DT.float32, name="warm_ps")
            for _ in range(8):
                nc.tensor.matmul(warm_ps[:], lhsT=wlhs[:], rhs=wrhs[:],
                                 start=True, stop=True,
                                 perf_mode=mybir.MatmulPerfMode.DoubleRow)
            # counts: accumulate the one-hots on the DVE (keeps the serial
            # PE weight-load/matmul chain to one matmul per tile pair)
            oh_acc = cpool.tile([128, FB, NUM], DT.float32)
            nc.vector.memset(oh_acc[:], 0.0)
            for bt, bn in FS_BATCHES:
                fs_b = fsp.tile([128, bn, 2 * D], DT.float8e4, name="fs_b")
                nc.sync.dma_start(out=fs_b[:], in_=x12f[:, bt:bt + bn, :])
                # one-hot labels for the whole batch in a single DVE op
                oh_b = fsp.tile([128, bn, NUM], DT.float8e4, name="oh_b")
                nc.vector.tensor_tensor(
                    out=oh_b[:],
                    in0=yf_sb[:, bt:bt + bn, :].to_broadcast([128, bn, NUM]),
                    in1=iota_f[:].to_broadcast([128, bn, NUM]),
                    op=ALU.is_equal)
                nc.vector.tensor_tensor(
                    out=oh_acc[:, 0:bn, :], in0=oh_acc[:, 0:bn, :],
                    in1=oh_b[:], op=ALU.add)
                for k in range(0, bn, 2):
                    t = bt + k
                    st, sp = (t == 0), (t == S_TILES - 2)
                    nc.tensor.matmul(
                        cs_ds_ps[:], lhsT=oh_b[:, k:k + 2, :],
                        rhs=fs_b[:, k:k + 2, :], start=st, stop=sp,
                        perf_mode=mybir.MatmulPerfMode.DoubleRow)
            # fold the FB slots, then one [128,16]x[128,1] matmul -> counts
            oh_slot = cpool.tile([128, NUM, 1], DT.float32)
            nc.vector.tensor_reduce(
                out=oh_slot[:],
                in_=oh_acc[:].rearrange("p s c -> p c s"),
                axis=mybir.AxisListType.X, op=ALU.add)
            ones_f = cpool.tile([128, 1], DT.float32)
            nc.vector.memset(ones_f[:], 1.0)
            nc.tensor.matmul(cnt_ps[:], lhsT=oh_slot[:, :, 0],
                             rhs=ones_f[:], start=True, stop=True)

            # ---- phase 2: class stats (all on 16 partitions) ----
            sums = cpool.tile([NUM, 2 * D], DT.float32)
            nc.vector.tensor_copy(sums[:], cs_ds_ps[:])
            cnt_sb = cpool.tile([NUM, 1], DT.float32)
            nc.vector.tensor_copy(cnt_sb[:], cnt_ps[:])

        rc = _emit_recip(nc, cpool, cnt_sb[:], [NUM, 1], "rc")
        centers = cpool.tile([NUM, D], DT.float32)
        nc.vector.tensor_scalar_mul(centers[:], sums[:, 0:D], rc[:])
        dsum = cpool.tile([NUM, D], DT.float32)
        nc.vector.tensor_tensor(
            out=dsum[:], in0=sums[:, D:2 * D], in1=sums[:, 0:D], op=ALU.subtract)
        nc.vector.tensor_scalar_mul(delta_bf[:, 0:D], dsum[:], rc[:])
        nc.vector.memset(delta_bf[:, D:D + 1], 1.0)
        csq = cpool.tile([NUM, D], DT.float32)
        nc.vector.tensor_tensor(
            out=csq[:], in0=centers[:], in1=centers[:], op=ALU.mult)
        csum = cpool.tile([NUM, 1], DT.float32)
        nc.vector.tensor_reduce(
            out=csum[:], in_=csq[:], axis=mybir.AxisListType.X, op=ALU.add)
        cinv = _emit_rsqrt(nc, cpool, csum[:], [NUM, 1], "cinv")
        # cn scaled by XS to keep fp8 values in the normal range; the
        # XS^2 factor on qq is undone by the exp scale below
        cinv16 = cpool.tile([NUM, 1], DT.float32)
        nc.vector.tensor_scalar(out=cinv16[:], in0=cinv[:], scalar1=XS,
                                scalar2=None, op0=ALU.mult)
        cn_f = cpool.tile([NUM, D], DT.float32)
        nc.vector.tensor_scalar_mul(cn_f[:], centers[:], cinv16[:])
        with tc.tile_pool(name="cnp", bufs=1, space="PSUM") as cnps:
            for h in range(2):
                tpc = cnps.tile([128, NUM], DT.float32, name=f"tpc{h}")
                nc.tensor.transpose(
                    tpc[:], in_=cn_f[:, h * 128:(h + 1) * 128],
                    identity=ident_f[0:NUM, 0:NUM])
                nc.vector.tensor_copy(cnT_sb[:, h, :], tpc[:])

        # ---- phase 3: main loop over table row blocks ----
        with tc.tile_pool(name="mi", bufs=8) as mpool, \
             tc.tile_pool(name="mo", bufs=5) as opool, \
             tc.tile_pool(name="mt", bufs=6) as tpool, \
             tc.tile_pool(name="mq", bufs=2, space="PSUM") as qps, \
             tc.tile_pool(name="mf", bufs=3, space="PSUM") as fps:
            sched = [(i * 2048, 2048) for i in range(12)] + [(R_PAD - 512, 512)]
            for off, nrows in sched:
                nt = nrows // 128
                gb = min(G, nt)          # tail block packs fewer rows/slot
                ns = nt // gb
                xgT_blk = mpool.tile([128, 2, nrows], DT.float8e4, name="xgT_blk")
                for h in range(2):
                    nc.sync.dma_start(out=xgT_blk[:, h, :],
                                      in_=xhT[h, :, off:off + nrows])
                x_blk = mpool.tile([128, ns, gb * D], DT.bfloat16, name="x_blk")
                nsplit = 2 if ns % 2 == 0 else 1
                nh = nrows // nsplit
                nsh = ns // nsplit
                for hh in range(nsplit):
                    nc.sync.dma_start(
                        out=x_blk[:, hh * nsh:(hh + 1) * nsh, :],
                        in_=xraw[off + hh * nh:off + (hh + 1) * nh, :].rearrange(
                            "(js p g) d -> p js (g d)", p=128, g=gb))
                out_blk = opool.tile([128, ns, gb * D], DT.bfloat16,
                                     name="out_blk")
                # softmax 1/denominator as exp(-ln(den)) on the Scalar
                # engine (ln+exp+copy live in one ACT table set) -- keeps
                # the DVE down to one fused op per row tile.
                for sb in range(nrows // 512):
                    qq = qps.tile([NUM, 512], DT.float32, name="qq")
                    nc.tensor.matmul(
                        qq[:], lhsT=cnT_sb[:],
                        rhs=xgT_blk[:, :, sb * 512:(sb + 1) * 512],
                        start=True, stop=True,
                        perf_mode=mybir.MatmulPerfMode.DoubleRow)
                    e4 = tpool.tile([NUM, 512], DT.bfloat16, name="e4")
                    nc.scalar.activation(out=e4[:], in_=qq[:], func=ACTF.Exp,
                                         scale=1.0 / (XS * XS))
                    # two fo tiles share one 2-bank PSUM allocation so a
                    # single strided DVE reciprocal covers both softmax
                    # denominators (no per-tile copies on any engine)
                    fos, rses = [], []
                    for half in range(2):
                        fo2 = fps.tile([128, 2, 512], DT.float32, name="fo2")
                        for k in range(2):
                            t4 = half * 2 + k
                            nc.tensor.matmul(
                                fo2[:, k, 0:D + 1],
                                lhsT=e4[:, t4 * 128:(t4 + 1) * 128],
                                rhs=delta_bf[:], start=True, stop=True)
                        rse2 = tpool.tile([128, 2], DT.float32,
                                          name=f"rse{half}")
                        nc.vector.reciprocal(rse2[:], fo2[:, :, D])
                        fos.append(fo2)
                        rses.append(rse2)
                    # finalize: tiles 0,2 via fused DVE op; tiles 1,3
                    # scaled on the Scalar engine, then ONE strided bf16
                    # add covers both (2x DVE mode, halves op overhead)
                    sc2 = tpool.tile([128, 2, D], DT.bfloat16, name="sc2")
                    for t4 in (0, 2):
                        j = sb * 4 + t4
                        js, g = j // gb, j % gb
                        nc.vector.scalar_tensor_tensor(
                            out=out_blk[:, js, g * D:(g + 1) * D],
                            in0=fos[t4 // 2][:, t4 % 2, 0:D],
                            scalar=rses[t4 // 2][:, t4 % 2:t4 % 2 + 1],
                            in1=x_blk[:, js, g * D:(g + 1) * D],
                            op0=ALU.mult, op1=ALU.add)
                    for i, t4 in enumerate((1, 3)):
                        nc.scalar.mul(sc2[:, i, :], fos[t4 // 2][:, t4 % 2, 0:D],
                                      rses[t4 // 2][:, t4 % 2:t4 % 2 + 1])
                    j1 = sb * 4 + 1
                    js1, g1 = j1 // gb, j1 % gb
                    # tiles 1 and 3 sit two D-slots apart in the same js slot
                    xs_ap = x_blk[:, js1, :].rearrange(
                        "p (s d) -> p s d", d=D)[:, g1:g1 + 3:2, :]
                    os_ap = out_blk[:, js1, :].rearrange(
                        "p (s d) -> p s d", d=D)[:, g1:g1 + 3:2, :]
                    nc.vector.tensor_tensor(
                        out=os_ap, in0=sc2[:], in1=xs_ap, op=ALU.add)
                # stores go out the gpsimd software-DGE ring (Q0) so they
                # never head-of-line block the Q1 prefetch stream
                for hh in range(nsplit):
                    nc.gpsimd.dma_start(
                        out=out[off + hh * nh:off + (hh + 1) * nh, :].rearrange(
                            "(js p g) d -> p js (g d)", p=128, g=gb),
                        in_=out_blk[:, hh * nsh:(hh + 1) * nsh, :])
    nc.finalize()
    return nc


def _shard_inputs(Q1_x, Q2_x, Q1_y, selected_idxes, remaining_idxes):
    """Host-side sharding/layout prep (slicing, normalize, transpose, cast)."""
    import ml_dtypes
    bf16 = ml_dtypes.bfloat16
    fp8 = ml_dtypes.float8_e4m3

    Q1_x = np.asarray(Q1_x, dtype=np.float32)
    Q2_x = np.asarray(Q2_x, dtype=np.float32)
    y = np.asarray(Q1_y).astype(np.float32)
    sel = np.asarray(selected_idxes).astype(np.int64)

    # few-shot block, partition-major, replicated to every core
    x12 = np.zeros((S_PAD, 2 * D), dtype=np.float32)
    x12[:S, 0:D] = Q1_x[sel]
    x12[:S, D:2 * D] = Q2_x[sel]
    x12 = np.ascontiguousarray(
        x12.reshape(S_TILES, 128, 2 * D).transpose(1, 0, 2)).astype(fp8)
    yv = np.full((S_PAD,), -1.0, dtype=np.float32)
    yv[:S] = y[sel]
    yf = np.ascontiguousarray(yv.reshape(S_TILES, 128).T)  # [128, S_TILES]

    norms = np.maximum(np.sqrt((Q1_x * Q1_x).sum(axis=1, keepdims=True)), 1e-8)
    xhat = Q1_x * (np.float32(XS) / norms)

    # xhatT column order: pi(t*128+q) = (t//G)*G*128 + q*G + (t%G), so the
    # row-major x/out tiles pack G consecutive DRAM rows per partition slot
    t = np.arange(R_PAD)
    tt, q = t // 128, t % 128
    pi = (tt // G) * G * 128 + q * G + (tt % G)
    tail = tt >= 192            # 512-row tail block uses G=4
    pi[tail] = 24576 + q[tail] * 4 + (tt[tail] - 192)

    in_maps = []
    for c in range(CORES):
        sl = slice(c * SLICE, (c + 1) * SLICE)
        xh_pad = np.zeros((R_PAD, D), dtype=np.float32)
        xh_pad[:SLICE] = xhat[sl]
        xhT = np.ascontiguousarray(
            xh_pad[pi].T.reshape(2, 128, R_PAD)).astype(fp8)
        xr_pad = np.zeros((R_PAD, D), dtype=np.float32)
        xr_pad[:SLICE] = Q1_x[sl]
        in_maps.append({
            "xhT": xhT,
            "xraw": xr_pad.astype(bf16),
            "x12f": x12,
            "yf": yf,
        })
    return in_maps


def kernel(Q1_x, Q2_x, Q1_y, selected_idxes, remaining_idxes, num, _bench=None):
    from concourse.bass_utils import run_bass_kernel_spmd

    in_maps = _shard_inputs(Q1_x, Q2_x, Q1_y, selected_idxes, remaining_idxes)
    nc = build_nc()
    kwargs = dict(_bench or {})
    res = run_bass_kernel_spmd(nc, in_maps, core_ids=list(range(CORES)), **kwargs)
    full = np.concatenate(
        [np.asarray(res.results[c]["out"][:SLICE]) for c in range(CORES)], axis=0)
    rem = np.asarray(remaining_idxes).astype(np.int64)
    out = full[rem].astype(np.float32)
    if _bench is not None:
        kernel.last_results = res
    return out
